# revision 32
# baseline (speedup 1.0000x reference)
"""Trainium2 Bass kernel for nn_MihGNNEmbeddingTest3 (gnn_message_passing).

Reference math:
    H = mlp(A_s @ emb)          (mlp = 3 linear layers, no activations)
    out[e] = relu(|<H[src_e], H[dst_e]>| / (||H[src_e]|| ||H[dst_e]||))

Since the mlp is affine, fold it:  H = A_s @ (emb @ W_eff^T) + b_eff.
Device work per core (node-sharded):  H_c = A_s[rows_c] @ E2 + b_eff
(E2 = emb @ W_eff^T precomputed on host), chunked AllGather of H
overlapping the matmul, then bulk dma_gather row fetches + fused
dot/norm reductions per edge.

Sharding: A_s rows (and nodes) split 1024/core across 8 cores; edges
split 1024/core. A_s shard is shipped pre-transposed in bf16 so k-tiles
land directly as matmul lhsT weights.
"""

import os
import sys

import numpy as np

try:
    import concourse.bass  # noqa: F401
except ImportError:  # pragma: no cover - grading env should have PYTHONPATH set
    for p in ("/opt/trn_rl_repo", "/root/.axon_site/_ro/trn_rl_repo"):
        if os.path.isdir(p) and p not in sys.path:
            sys.path.insert(0, p)

import ml_dtypes

N, D, B = 8192, 256, 8192
N_CORES = 8
ROWS = N // N_CORES  # A_s rows / nodes per core
EPC = B // N_CORES   # edges per core
KT = N // 128        # contraction tiles
MT = ROWS // 128     # output row tiles per core
JT = EPC // 128      # edge blocks per core

_CACHE = {}
LAST_RESULTS = None  # BassKernelResults of the most recent run (for test.py)


def _build():
    import concourse.bacc as bacc
    import concourse.bass as bass
    import concourse.mybir as mybir
    import concourse.tile as tile

    fp32 = mybir.dt.float32
    bf16 = mybir.dt.bfloat16

    nc = bacc.Bacc(num_devices=N_CORES)
    # partition-major layouts: [p, k_tile, cols] so each DMA chunk reads
    # large contiguous per-partition spans from DRAM; at split in row halves
    # so m-group A's data arrives first
    ata = nc.declare_dram_parameter("ata", [128, KT, ROWS // 2], bf16, isOutput=False)
    atb = nc.declare_dram_parameter("atb", [128, KT, ROWS // 2], bf16, isOutput=False)
    e2 = nc.declare_dram_parameter("e2", [128, KT, D], bf16, isOutput=False)
    # bias for the H^T layout, pre-broadcast on host:
    # biastf[p, d, :] = b_eff[d*128 + p]
    biastf = nc.declare_dram_parameter(
        "biastf", [128, 2, ROWS // 2], fp32, isOutput=False
    )
    ident = nc.declare_dram_parameter("ident", [128, 128], bf16, isOutput=False)
    # dma_gather index layout: idx i lives at [i % 16, i // 16], 16-row
    # pattern replicated to fill 128 partitions. Three calls:
    #   cols  0:16  "early"  [src|dst] of edges 0..127 (rows all in AG0 half)
    #   cols 16:80  "A"      [src|dst] of edges 128..639
    #   cols 80:128 "B"      [src|dst] of edges 640..1023
    gidx = nc.declare_dram_parameter(
        "gidx", [128, EPC // 8], mybir.dt.int16, isOutput=False
    )
    out = nc.declare_dram_parameter("out", [128, JT], fp32, isOutput=True)

    with tile.TileContext(nc) as tc:
        with (
            tc.tile_pool(name="atp", bufs=1) as atp,
            tc.tile_pool(name="e2p", bufs=1) as e2p,
            tc.tile_pool(name="psum", bufs=1, space="PSUM") as psum,
            tc.tile_pool(name="hsb", bufs=4) as hsbp,
            tc.tile_pool(name="dram", bufs=1, space="DRAM") as dram,
            tc.tile_pool(name="const", bufs=1) as constp,
            tc.tile_pool(name="gat", bufs=1) as gat,
            tc.tile_pool(name="small", bufs=1) as small,
        ):
            h_shard_a = dram.tile([ROWS // 2, D], bf16)
            h_shard_b = dram.tile([ROWS // 2, D], bf16)
            h_shards = [h_shard_a, h_shard_b]
            h_full = dram.tile([N, D], bf16)

            # Batched loads: few big DMAs with 8-16KB contiguous descriptors.
            # Small leading chunks so the first matmuls start early. e2 goes
            # on the scalar HWDGE ring so it arrives in parallel with the
            # first at chunks on the sync ring.
            AT_BOUNDS = [0, 2, 8, 16, 24, 32, 40, 48, 56, 64]
            E2_BOUNDS = [0, 4, 16, 32, 48, 64]
            at_t = [[None] * KT, [None] * KT]  # [group][k]
            e2_t = [None] * KT

            def load_e2(ci):
                lo, hi = E2_BOUNDS[ci], E2_BOUNDS[ci + 1]
                ec = e2p.tile([128, hi - lo, D], bf16, name=f"e2c_{ci}", tag=f"e2c{ci}")
                nc.scalar.dma_start(out=ec[:], in_=e2[:, lo:hi, :])
                for k in range(lo, hi):
                    e2_t[k] = ec[:, k - lo, :]

            def load_at(ci, g):
                src = ata if g == 0 else atb
                lo, hi = AT_BOUNDS[ci], AT_BOUNDS[ci + 1]
                ac = atp.tile(
                    [128, hi - lo, ROWS // 2], bf16,
                    name=f"atc_{g}_{ci}", tag=f"atc{g}_{ci}",
                )
                nc.sync.dma_start(out=ac[:], in_=src[:, lo:hi, :])
                for k in range(lo, hi):
                    at_t[g][k] = ac[:, k - lo, :]

            # group A (row half 0) + e2 first, then group B
            load_e2(0)
            load_at(0, 0)
            load_at(1, 0)
            load_e2(1)
            load_at(2, 0)
            load_at(3, 0)
            load_e2(2)
            load_at(4, 0)
            load_at(5, 0)
            load_e2(3)
            load_at(6, 0)
            load_at(7, 0)
            load_e2(4)
            load_at(8, 0)
            for ci in range(9):
                load_at(ci, 1)

            biastf_sb = constp.tile([128, 2, ROWS // 2], fp32)
            nc.scalar.dma_start(out=biastf_sb[:], in_=biastf[:])
            ident_sb = constp.tile([128, 128], bf16)
            nc.scalar.dma_start(out=ident_sb[:], in_=ident[:])
            gidx_sb = constp.tile([128, EPC // 8], mybir.dt.int16)
            nc.scalar.dma_start(out=gidx_sb[:], in_=gidx[:])
            out_sb = constp.tile([128, JT], fp32)

            # H^T matmul: lhsT = e2 k-tile d-half (stationary), rhs = A^T
            # k-tile (moving, 512 wide) -> psum [128(d), 512(rows)]. 256 big
            # matmuls instead of 512 small ones (fewer LDWEIGHTS stalls),
            # and each 512-row group finishes sooner, so its AllGather
            # chunk triggers earlier.
            RH = ROWS // 2  # rows per group
            with nc.named_scope("matmul"):
                for g in range(2):
                    psT = [
                        psum.tile([128, RH], fp32, name=f"psT_{g}_{d}", tag=f"psT{d}")
                        for d in range(2)
                    ]
                    for k in range(KT):
                        for d in range(2):
                            nc.tensor.matmul(
                                out=psT[d][:],
                                lhsT=e2_t[k][:, d * 128:(d + 1) * 128],
                                rhs=at_t[g][k][:],
                                start=(k == 0),
                                stop=(k == KT - 1),
                            )
                    hT = []
                    for d in range(2):
                        t = hsbp.tile([128, RH], bf16, name=f"hT_{g}_{d}", tag=f"hT{d}")
                        nc.vector.tensor_tensor(
                            out=t[:],
                            in0=psT[d][:],
                            in1=biastf_sb[:, d, :],
                            op=mybir.AluOpType.add,
                        )
                        hT.append(t)
                    # back to row-major via PE transpose (in_.T @ I into
                    # PSUM bf16), DVE copy to SBUF, store to the AG input
                    for j in range(RH // 128):
                        hr = hsbp.tile([128, D], bf16, name=f"hr_{g}_{j}", tag=f"hr{j % 2}")
                        for d in range(2):
                            pst = psum.tile(
                                [128, 128], bf16,
                                name=f"pst_{g}_{j}_{d}", tag=f"pst{(j * 2 + d) % 2}",
                            )
                            nc.tensor.transpose(
                                out=pst[:],
                                in_=hT[d][:, j * 128:(j + 1) * 128],
                                identity=ident_sb[:],
                            )
                            nc.vector.tensor_copy(
                                out=hr[:, d * 128:(d + 1) * 128], in_=pst[:]
                            )
                        nc.scalar.dma_start(
                            out=h_shards[g][j * 128:(j + 1) * 128, :], in_=hr[:]
                        )
                    with nc.named_scope(f"allgather{g}"):
                        # chunk g: rows [g*512, (g+1)*512) of every core's
                        # shard -> h_full rows [g*4096 + core*512 ...)
                        nc.gpsimd.collective_compute(
                            "AllGather",
                            mybir.AluOpType.bypass,
                            replica_groups=[list(range(N_CORES))],
                            ins=[h_shards[g][:]],
                            outs=[h_full[g * N // 2:(g + 1) * N // 2, :]],
                        )

            with nc.named_scope("edges"):
                # merged [src|dst] gathers. The "early" call reads only the
                # AG0 half of h_full, so it (and its math) overlaps AG1.
                hsd_e = gat.tile([128, 2, D], bf16, name="hsd_e", tag="hsde")
                hsd_a = gat.tile([128, 8, D], bf16, name="hsd_a", tag="hsda")
                hsd_b = gat.tile([128, 6, D], bf16, name="hsd_b", tag="hsdb")
                dot = small.tile([128, JT], fp32, name="dot", tag="dot")
                ns = small.tile([128, JT], fp32, name="ns", tag="ns")
                nd = small.tile([128, JT], fp32, name="nd", tag="nd")
                nc.gpsimd.dma_gather(
                    out_ap=hsd_e[:],
                    in_ap=h_full[0:N // 2, :],
                    idxs_ap=gidx_sb[:, 0:16],
                    num_idxs=256,
                    num_idxs_reg=256,
                    elem_size=D,
                )
                nc.gpsimd.dma_gather(
                    out_ap=hsd_a[:],
                    in_ap=h_full[:],
                    idxs_ap=gidx_sb[:, 16:80],
                    num_idxs=1024,
                    num_idxs_reg=1024,
                    elem_size=D,
                )
                nc.gpsimd.dma_gather(
                    out_ap=hsd_b[:],
                    in_ap=h_full[:],
                    idxs_ap=gidx_sb[:, 80:128],
                    num_idxs=768,
                    num_idxs_reg=768,
                    elem_size=D,
                )
                for j in range(JT):
                    if j == 0:
                        hs = hsd_e[:, 0, :]
                        hd = hsd_e[:, 1, :]
                    elif j <= 4:
                        hs = hsd_a[:, j - 1, :]
                        hd = hsd_a[:, 4 + j - 1, :]
                    else:
                        hs = hsd_b[:, j - 5, :]
                        hd = hsd_b[:, 3 + j - 5, :]
                    prod = gat.tile([128, D], fp32, name=f"prod_{j}", tag=f"prod{j % 2}")
                    sq_s = gat.tile([128, D], fp32, name=f"sq_s_{j}", tag=f"sq_s{j % 2}")
                    sq_d = gat.tile([128, D], fp32, name=f"sq_d_{j}", tag=f"sq_d{j % 2}")
                    nc.vector.tensor_tensor(
                        out=prod[:], in0=hs, in1=hd,
                        op=mybir.AluOpType.mult,
                    )
                    nc.vector.tensor_reduce(
                        out=dot[:, j:j + 1], in_=prod[:], axis=mybir.AxisListType.X,
                        op=mybir.AluOpType.add,
                    )
                    nc.scalar.square(sq_s[:], hs)
                    nc.scalar.square(sq_d[:], hd)
                    nc.vector.tensor_reduce(
                        out=ns[:, j:j + 1], in_=sq_s[:], axis=mybir.AxisListType.X,
                        op=mybir.AluOpType.add,
                    )
                    nc.vector.tensor_reduce(
                        out=nd[:, j:j + 1], in_=sq_d[:], axis=mybir.AxisListType.X,
                        op=mybir.AluOpType.add,
                    )
                nsnd = small.tile([128, JT], fp32, name="nsnd", tag="nsnd")
                nc.vector.tensor_tensor(
                    out=nsnd[:], in0=ns[:], in1=nd[:], op=mybir.AluOpType.mult
                )
                st = small.tile([128, JT], fp32, name="st", tag="st")
                nc.scalar.sqrt(st[:], nsnd[:])
                inv = small.tile([128, JT], fp32, name="inv", tag="inv")
                nc.vector.reciprocal(inv[:], st[:])
                ad = small.tile([128, JT], fp32, name="ad", tag="ad")
                nc.vector.tensor_scalar(
                    out=ad[:].bitcast(mybir.dt.uint32),
                    in0=dot[:].bitcast(mybir.dt.uint32),
                    scalar1=0x7FFFFFFF, scalar2=None,
                    op0=mybir.AluOpType.bitwise_and,
                )
                nc.vector.tensor_tensor(
                    out=out_sb[:],
                    in0=ad[:],
                    in1=inv[:],
                    op=mybir.AluOpType.mult,
                )

            nc.sync.dma_start(out=out[:], in_=out_sb[:])

    nc.compile()
    return nc


def _get_nc():
    if "nc" not in _CACHE:
        _CACHE["nc"] = _build()
    return _CACHE["nc"]


def kernel(edges, A_s, emb, Ws, bs):
    global LAST_RESULTS
    from concourse.bass_utils import run_bass_kernel_spmd

    bf16 = ml_dtypes.bfloat16
    A = np.asarray(A_s, dtype=np.float32)
    E = np.asarray(emb, dtype=np.float32)
    W = np.asarray(Ws, dtype=np.float32)
    b = np.asarray(bs, dtype=np.float32)
    ed = np.asarray(edges)

    M = W[0].T @ W[1].T @ W[2].T                      # [D, D]
    # partition-major: [128(p), KT(t), D] with row t*128+p at [p, t, :]
    E2 = np.ascontiguousarray(
        (E @ M).astype(bf16).reshape(KT, 128, D).transpose(1, 0, 2)
    )
    b_eff = (b[0] @ W[1].T + b[1]) @ W[2].T + b[2]    # [D]
    biastf_host = np.ascontiguousarray(
        np.broadcast_to(
            b_eff.astype(np.float32).reshape(2, 128).T[:, :, None],
            (128, 2, ROWS // 2),
        )
    )
    ident_host = np.eye(128, dtype=bf16)

    def remap(n):
        # node id -> h_full row (2-chunk AllGather layout)
        o = n // ROWS
        l = n % ROWS
        g = l // (ROWS // 2)
        return g * (N // 2) + o * (ROWS // 2) + (l % (ROWS // 2))

    in_maps = []
    perms = []
    for c in range(N_CORES):
        at_full = (
            A[c * ROWS:(c + 1) * ROWS, :].T.astype(bf16)  # [N, ROWS]
            .reshape(KT, 128, ROWS).transpose(1, 0, 2)    # [128, KT, ROWS]
        )
        ata_c = np.ascontiguousarray(at_full[:, :, :ROWS // 2])
        atb_c = np.ascontiguousarray(at_full[:, :, ROWS // 2:])
        e = ed[c * EPC:(c + 1) * EPC].astype(np.int64)

        def pack_idx(arr):
            # dma_gather index layout: idx i at [i % 16, i // 16], 16-row
            # pattern tiled to 128 partitions
            t = arr.astype(np.int16).reshape(len(arr) // 16, 16).T
            return np.tile(t, (8, 1))

        s_r = remap(e[:, 0])
        d_r = remap(e[:, 1])
        # early bucket: both rows in AG0's half of h_full -> slot j-tile 0
        early = np.nonzero((s_r < N // 2) & (d_r < N // 2))[0]
        assert len(early) >= 128, f"early bucket too small: {len(early)}"
        rest = np.setdiff1d(np.arange(EPC), early[:128], assume_unique=False)
        perm = np.concatenate([early[:128], rest])
        perms.append(perm)
        s_p, d_p = s_r[perm], d_r[perm]
        gidx_c = np.ascontiguousarray(np.concatenate(
            [
                pack_idx(np.concatenate([s_p[0:128], d_p[0:128]])),
                pack_idx(np.concatenate([s_p[128:640], d_p[128:640]])),
                pack_idx(np.concatenate([s_p[640:1024], d_p[640:1024]])),
            ],
            axis=1,
        ))
        in_maps.append(
            {"ata": ata_c, "atb": atb_c, "e2": E2, "biastf": biastf_host,
             "ident": ident_host, "gidx": gidx_c}
        )

    nc = _get_nc()
    kw = {}
    if os.environ.get("KERNEL_TRACE_KW"):
        import json
        kw = json.loads(os.environ["KERNEL_TRACE_KW"])
    res = run_bass_kernel_spmd(nc, in_maps, list(range(N_CORES)), **kw)
    LAST_RESULTS = res

    outs = []
    for c in range(N_CORES):
        o_perm = np.ascontiguousarray(res.results[c]["out"].T).reshape(-1)
        o = np.empty_like(o_perm)
        o[perms[c]] = o_perm
        outs.append(o)
    out = np.concatenate(outs)
    return np.maximum(out, 0.0).astype(np.float32)


# revision 33
# speedup vs baseline: 1.1240x; 1.1240x over previous
"""Trainium2 Bass kernel for nn_MihGNNEmbeddingTest3 (gnn_message_passing).

Reference math:
    H = mlp(A_s @ emb)          (mlp = 3 linear layers, no activations)
    out[e] = relu(|<H[src_e], H[dst_e]>| / (||H[src_e]|| ||H[dst_e]||))

Since the mlp is affine, fold it:  H = A_s @ (emb @ W_eff^T) + b_eff.
Device work per core (node-sharded):  H_c = A_s[rows_c] @ E2 + b_eff
(E2 = emb @ W_eff^T precomputed on host), chunked AllGather of H
overlapping the matmul, then bulk dma_gather row fetches + fused
dot/norm reductions per edge.

Sharding: A_s rows (and nodes) split 1024/core across 8 cores; edges
split 1024/core. A_s shard is shipped pre-transposed in bf16 so k-tiles
land directly as matmul lhsT weights.
"""

import os
import sys

import numpy as np

try:
    import concourse.bass  # noqa: F401
except ImportError:  # pragma: no cover - grading env should have PYTHONPATH set
    for p in ("/opt/trn_rl_repo", "/root/.axon_site/_ro/trn_rl_repo"):
        if os.path.isdir(p) and p not in sys.path:
            sys.path.insert(0, p)

import ml_dtypes

N, D, B = 8192, 256, 8192
N_CORES = 8
ROWS = N // N_CORES  # A_s rows / nodes per core
EPC = B // N_CORES   # edges per core
KT = N // 128        # contraction tiles
MT = ROWS // 128     # output row tiles per core
JT = EPC // 128      # edge blocks per core

_CACHE = {}
LAST_RESULTS = None  # BassKernelResults of the most recent run (for test.py)


def _build():
    import concourse.bacc as bacc
    import concourse.bass as bass
    import concourse.mybir as mybir
    import concourse.tile as tile

    fp32 = mybir.dt.float32
    bf16 = mybir.dt.bfloat16

    nc = bacc.Bacc(num_devices=N_CORES)
    # partition-major layouts: [p, k_tile, cols] so each DMA chunk reads
    # large contiguous per-partition spans from DRAM; at split in row halves
    # so m-group A's data arrives first
    ata = nc.declare_dram_parameter("ata", [128, KT, ROWS // 2], bf16, isOutput=False)
    atb = nc.declare_dram_parameter("atb", [128, KT, ROWS // 2], bf16, isOutput=False)
    e2 = nc.declare_dram_parameter("e2", [128, KT, D], bf16, isOutput=False)
    # bias for the H^T layout, pre-broadcast on host:
    # biastf[p, d, :] = b_eff[d*128 + p]
    biastf = nc.declare_dram_parameter(
        "biastf", [128, 2, ROWS // 2], fp32, isOutput=False
    )
    ident = nc.declare_dram_parameter("ident", [128, 128], bf16, isOutput=False)
    # dma_gather index layout: idx i lives at [i % 16, i // 16], 16-row
    # pattern replicated to fill 128 partitions. Three calls:
    #   cols  0:16  "early"  [src|dst] of edges 0..127 (rows all in AG0 half)
    #   cols 16:80  "A"      [src|dst] of edges 128..639
    #   cols 80:128 "B"      [src|dst] of edges 640..1023
    gidx = nc.declare_dram_parameter(
        "gidx", [128, EPC // 8], mybir.dt.int16, isOutput=False
    )
    out = nc.declare_dram_parameter("out", [128, JT], fp32, isOutput=True)

    with tile.TileContext(nc) as tc:
        with (
            tc.tile_pool(name="atp", bufs=1) as atp,
            tc.tile_pool(name="e2p", bufs=1) as e2p,
            tc.tile_pool(name="psum", bufs=1, space="PSUM") as psum,
            tc.tile_pool(name="hsb", bufs=4) as hsbp,
            tc.tile_pool(name="dram", bufs=1, space="DRAM") as dram,
            tc.tile_pool(name="const", bufs=1) as constp,
            tc.tile_pool(name="gat", bufs=1) as gat,
            tc.tile_pool(name="small", bufs=1) as small,
        ):
            h_shard_a = dram.tile([ROWS // 2, D], bf16)
            h_shard_b = dram.tile([ROWS // 2, D], bf16)
            h_shards = [h_shard_a, h_shard_b]
            h_full = dram.tile([N, D], bf16)

            # Batched loads: few big DMAs with 8-16KB contiguous descriptors.
            # Small leading chunks so the first matmuls start early. e2 goes
            # on the scalar HWDGE ring so it arrives in parallel with the
            # first at chunks on the sync ring.
            AT_BOUNDS = [0, 2, 8, 16, 24, 32, 40, 48, 56, 64]
            E2_BOUNDS = [0, 4, 16, 32, 48, 64]
            at_t = [[None] * KT, [None] * KT]  # [group][k]
            e2_t = [None] * KT

            def load_e2(ci):
                lo, hi = E2_BOUNDS[ci], E2_BOUNDS[ci + 1]
                ec = e2p.tile([128, hi - lo, D], bf16, name=f"e2c_{ci}", tag=f"e2c{ci}")
                nc.scalar.dma_start(out=ec[:], in_=e2[:, lo:hi, :])
                for k in range(lo, hi):
                    e2_t[k] = ec[:, k - lo, :]

            def load_at(ci, g, eng=None):
                src = ata if g == 0 else atb
                lo, hi = AT_BOUNDS[ci], AT_BOUNDS[ci + 1]
                ac = atp.tile(
                    [128, hi - lo, ROWS // 2], bf16,
                    name=f"atc_{g}_{ci}", tag=f"atc{g}_{ci}",
                )
                (eng or nc.sync).dma_start(out=ac[:], in_=src[:, lo:hi, :])
                for k in range(lo, hi):
                    at_t[g][k] = ac[:, k - lo, :]

            # group A (row half 0) + e2 first, then group B
            load_e2(0)
            load_at(0, 0)
            load_at(1, 0)
            load_e2(1)
            load_at(2, 0)
            load_at(3, 0)
            load_e2(2)
            load_at(4, 0)
            load_at(5, 0)
            load_e2(3)
            load_at(6, 0)
            load_at(7, 0)
            load_e2(4)
            load_at(8, 0)
            for ci in range(5):
                load_at(ci, 1)

            biastf_sb = constp.tile([128, 2, ROWS // 2], fp32)
            nc.scalar.dma_start(out=biastf_sb[:], in_=biastf[:])
            ident_sb = constp.tile([128, 128], bf16)
            nc.scalar.dma_start(out=ident_sb[:], in_=ident[:])
            gidx_sb = constp.tile([128, EPC // 8], mybir.dt.int16)
            nc.scalar.dma_start(out=gidx_sb[:], in_=gidx[:])
            out_sb = constp.tile([128, JT], fp32)

            # H^T matmul: lhsT = e2 k-tile d-half (stationary), rhs = A^T
            # k-tile (moving, 512 wide) -> psum [128(d), 512(rows)]. 256 big
            # matmuls instead of 512 small ones (fewer LDWEIGHTS stalls),
            # and each 512-row group finishes sooner, so its AllGather
            # chunk triggers earlier.
            RH = ROWS // 2  # rows per group
            with nc.named_scope("matmul"):
                for g in range(2):
                    psT = [
                        psum.tile([128, RH], fp32, name=f"psT_{g}_{d}", tag=f"psT{d}")
                        for d in range(2)
                    ]
                    for k in range(KT):
                        for d in range(2):
                            nc.tensor.matmul(
                                out=psT[d][:],
                                lhsT=e2_t[k][:, d * 128:(d + 1) * 128],
                                rhs=at_t[g][k][:],
                                start=(k == 0),
                                stop=(k == KT - 1),
                            )
                    hT = []
                    for d in range(2):
                        t = hsbp.tile([128, RH], bf16, name=f"hT_{g}_{d}", tag=f"hT{d}")
                        nc.vector.tensor_tensor(
                            out=t[:],
                            in0=psT[d][:],
                            in1=biastf_sb[:, d, :],
                            op=mybir.AluOpType.add,
                        )
                        hT.append(t)
                    # back to row-major via PE transpose (in_.T @ I into
                    # PSUM bf16), DVE copy to SBUF, store to the AG input
                    for j in range(RH // 128):
                        hr = hsbp.tile([128, D], bf16, name=f"hr_{g}_{j}", tag=f"hr{j % 2}")
                        for d in range(2):
                            pst = psum.tile(
                                [128, 128], bf16,
                                name=f"pst_{g}_{j}_{d}", tag=f"pst{(j * 2 + d) % 2}",
                            )
                            nc.tensor.transpose(
                                out=pst[:],
                                in_=hT[d][:, j * 128:(j + 1) * 128],
                                identity=ident_sb[:],
                            )
                            nc.vector.tensor_copy(
                                out=hr[:, d * 128:(d + 1) * 128], in_=pst[:]
                            )
                        nc.scalar.dma_start(
                            out=h_shards[g][j * 128:(j + 1) * 128, :], in_=hr[:]
                        )
                    if g == 0:
                        # tail atb chunks on the scalar ring, issued after
                        # the AG0 input stores so those complete first
                        for ci in range(5, 9):
                            load_at(ci, 1, eng=nc.scalar)
                    with nc.named_scope(f"allgather{g}"):
                        # chunk g: rows [g*512, (g+1)*512) of every core's
                        # shard -> h_full rows [g*4096 + core*512 ...)
                        nc.gpsimd.collective_compute(
                            "AllGather",
                            mybir.AluOpType.bypass,
                            replica_groups=[list(range(N_CORES))],
                            ins=[h_shards[g][:]],
                            outs=[h_full[g * N // 2:(g + 1) * N // 2, :]],
                        )

            with nc.named_scope("edges"):
                # two merged gathers: call c fetches [src | dst] rows for
                # edges [c*512, (c+1)*512) -> [128, 8, D] (cols 0-3 = hs
                # j-tiles, cols 4-7 = hd). Math on call 0 overlaps call 1's
                # descriptor generation on the Q7.
                hsd = [
                    gat.tile([128, JT, D], bf16, name=f"hsd_{c}", tag=f"hsd{c}")
                    for c in range(2)
                ]
                dot = small.tile([128, JT], fp32, name="dot", tag="dot")
                ns = small.tile([128, JT], fp32, name="ns", tag="ns")
                nd = small.tile([128, JT], fp32, name="nd", tag="nd")
                for c in range(2):
                    nc.gpsimd.dma_gather(
                        out_ap=hsd[c][:],
                        in_ap=h_full[:],
                        idxs_ap=gidx_sb[:, c * 64:(c + 1) * 64],
                        num_idxs=EPC,
                        num_idxs_reg=EPC,
                        elem_size=D,
                    )
                for j in range(JT):
                    c, jj = divmod(j, JT // 2)
                    hs = hsd[c][:, jj, :]
                    hd = hsd[c][:, JT // 2 + jj, :]
                    prod = gat.tile([128, D], fp32, name=f"prod_{j}", tag=f"prod{j % 2}")
                    sq_s = gat.tile([128, D], fp32, name=f"sq_s_{j}", tag=f"sq_s{j % 2}")
                    sq_d = gat.tile([128, D], fp32, name=f"sq_d_{j}", tag=f"sq_d{j % 2}")
                    nc.vector.tensor_tensor(
                        out=prod[:], in0=hs, in1=hd,
                        op=mybir.AluOpType.mult,
                    )
                    nc.vector.tensor_reduce(
                        out=dot[:, j:j + 1], in_=prod[:], axis=mybir.AxisListType.X,
                        op=mybir.AluOpType.add,
                    )
                    nc.scalar.square(sq_s[:], hs)
                    nc.scalar.square(sq_d[:], hd)
                    nc.vector.tensor_reduce(
                        out=ns[:, j:j + 1], in_=sq_s[:], axis=mybir.AxisListType.X,
                        op=mybir.AluOpType.add,
                    )
                    nc.vector.tensor_reduce(
                        out=nd[:, j:j + 1], in_=sq_d[:], axis=mybir.AxisListType.X,
                        op=mybir.AluOpType.add,
                    )
                nsnd = small.tile([128, JT], fp32, name="nsnd", tag="nsnd")
                nc.vector.tensor_tensor(
                    out=nsnd[:], in0=ns[:], in1=nd[:], op=mybir.AluOpType.mult
                )
                st = small.tile([128, JT], fp32, name="st", tag="st")
                nc.scalar.sqrt(st[:], nsnd[:])
                inv = small.tile([128, JT], fp32, name="inv", tag="inv")
                nc.vector.reciprocal(inv[:], st[:])
                ad = small.tile([128, JT], fp32, name="ad", tag="ad")
                nc.vector.tensor_scalar(
                    out=ad[:].bitcast(mybir.dt.uint32),
                    in0=dot[:].bitcast(mybir.dt.uint32),
                    scalar1=0x7FFFFFFF, scalar2=None,
                    op0=mybir.AluOpType.bitwise_and,
                )
                nc.vector.tensor_tensor(
                    out=out_sb[:],
                    in0=ad[:],
                    in1=inv[:],
                    op=mybir.AluOpType.mult,
                )

            nc.sync.dma_start(out=out[:], in_=out_sb[:])

    nc.compile()
    return nc


def _get_nc():
    if "nc" not in _CACHE:
        _CACHE["nc"] = _build()
    return _CACHE["nc"]


def kernel(edges, A_s, emb, Ws, bs):
    global LAST_RESULTS
    from concourse.bass_utils import run_bass_kernel_spmd

    bf16 = ml_dtypes.bfloat16
    A = np.asarray(A_s, dtype=np.float32)
    E = np.asarray(emb, dtype=np.float32)
    W = np.asarray(Ws, dtype=np.float32)
    b = np.asarray(bs, dtype=np.float32)
    ed = np.asarray(edges)

    M = W[0].T @ W[1].T @ W[2].T                      # [D, D]
    # partition-major: [128(p), KT(t), D] with row t*128+p at [p, t, :]
    E2 = np.ascontiguousarray(
        (E @ M).astype(bf16).reshape(KT, 128, D).transpose(1, 0, 2)
    )
    b_eff = (b[0] @ W[1].T + b[1]) @ W[2].T + b[2]    # [D]
    biastf_host = np.ascontiguousarray(
        np.broadcast_to(
            b_eff.astype(np.float32).reshape(2, 128).T[:, :, None],
            (128, 2, ROWS // 2),
        )
    )
    ident_host = np.eye(128, dtype=bf16)

    def remap(n):
        # node id -> h_full row (2-chunk AllGather layout)
        o = n // ROWS
        l = n % ROWS
        g = l // (ROWS // 2)
        return g * (N // 2) + o * (ROWS // 2) + (l % (ROWS // 2))

    in_maps = []
    for c in range(N_CORES):
        at_full = (
            A[c * ROWS:(c + 1) * ROWS, :].T.astype(bf16)  # [N, ROWS]
            .reshape(KT, 128, ROWS).transpose(1, 0, 2)    # [128, KT, ROWS]
        )
        ata_c = np.ascontiguousarray(at_full[:, :, :ROWS // 2])
        atb_c = np.ascontiguousarray(at_full[:, :, ROWS // 2:])
        e = ed[c * EPC:(c + 1) * EPC].astype(np.int64)

        def pack_idx(arr):
            # dma_gather index layout: idx i at [i % 16, i // 16], 16-row
            # pattern tiled to 128 partitions
            t = arr.astype(np.int16).reshape(len(arr) // 16, 16).T
            return np.tile(t, (8, 1))

        s_r = remap(e[:, 0])
        d_r = remap(e[:, 1])
        half = EPC // 2
        gidx_c = np.ascontiguousarray(np.concatenate(
            [
                pack_idx(np.concatenate([s_r[g * half:(g + 1) * half],
                                         d_r[g * half:(g + 1) * half]]))
                for g in range(2)
            ],
            axis=1,
        ))
        in_maps.append(
            {"ata": ata_c, "atb": atb_c, "e2": E2, "biastf": biastf_host,
             "ident": ident_host, "gidx": gidx_c}
        )

    nc = _get_nc()
    kw = {}
    if os.environ.get("KERNEL_TRACE_KW"):
        import json
        kw = json.loads(os.environ["KERNEL_TRACE_KW"])
    res = run_bass_kernel_spmd(nc, in_maps, list(range(N_CORES)), **kw)
    LAST_RESULTS = res

    out = np.concatenate(
        [np.ascontiguousarray(res.results[c]["out"].T).reshape(-1) for c in range(N_CORES)]
    )
    return np.maximum(out, 0.0).astype(np.float32)
